# revision 60
# baseline (speedup 1.0000x reference)
"""Single-head attention (B=8, S=2048, D=U=1024) on 8 TRN2 NeuronCores.

Sharding: data-parallel over batch — core b computes batch b end-to-end,
no cross-core communication.

v9 on top of v2's reassociated scores (scores/32 = x·M·x^T, M = WqWk^T/32):

1. Mask-sorted token permutation (host-side, exact): tokens are sorted
   unmasked-first per batch, so Et[k,q] = 0 for every (k >= 1152,
   q < 896) pair whenever n_unmasked in [896, 1152] (true w.p.
   1-1e-8 per batch; a no-skip variant is compiled on demand
   otherwise).  Phase E skips those score columns and phase F sums
   only k-tiles 0..8 for q-tiles 0..6 — exact, ~19% off both S×S
   stages.  The host inverse-permutes the output rows.
2. fp8e4 DoubleRow scores (phase E): T1 (x64, into e4m3 normal range)
   and the per-k-tile xT blocks are quantized to fp8e4; each (k-tile,
   512-col) score block is one pure-DR accumulation chain at 2x PE
   throughput (measured; CoreSim's 4x is optimistic).  NEVER switch
   perf mode inside one PSUM accumulation group — that wedges the
   exec unit.  The Exp epilogue folds the 1/64 with its scale arg; the
   rank-1 mask constant carries the 64x.  Adds ~1.6e-2 rel err
   (deterministic for fixed inputs), inside the 2e-2 budget.
3. Input DMA spread over three queues (~100-115 GB/s per queue
   regardless of DGE type): Wk+Wv on the casting SWDGE (gpsimd) queue;
   Wq pair-wise f32 on the ACT/SP HWDGE queues, cast f32->bf16 on ACT
   (idle until phase E); x f32 split even/odd across SP/ACT with ACT
   casts.  x chunks 0..8,10 are transposed inside phase B to fill the
   DMA-bound start.  Output tiles alternate queues; the last two split
   halves across both.
4. V projection in uq-outer 256-col quarters (2-deep wv ring, both
   first quarters prefetched) to overlap the SWDGE wv stream.

Rejected with measurements: M sharding via collectives (2MB AllReduce
takes ~87 us here — can't hide), 3-term fp8 residual splits and any
fp8 in ctx/V/T1T (error or cost), wk on HWDGE / other DMA orders.

Per-core pipeline (fp32 PSUM everywhere):
  A. Wk f32 --SWDGE cast--> bf16 chunks; Wq f32 --HWDGE--> ACT cast;
     both --PE transpose--> WkT/WqT [u,d].
  B. M[d1,d2] = sum_u WqT[u,d1] WkT[u,d2], scaled 1/32 in the epilogue;
     x chunks 0..8,10 transposed between accumulation blocks.
  C. remaining x transposes interleaved with T1T[d2,q] = sum_d1
     M[d1,d2] xT[d1,q]; epilogue scales by 64 into fp8e4 t1T8.
  D. V[s,u] = sum_d xT[d,s] Wv[d,u] + bv (uq-outer quarters).
  E. scoresT[k,q] = sum_d2 xT8[d2,k] t1T8[d2,q] via fp8 DoubleRow;
     rank-1 padding mask 64*c_k*m_q (c = -10000*(1-m)) via DVE
     scalar_tensor_tensor (skipped for all-unmasked k-tiles where
     c == 0); Et = exp(scoresT/64) on ACT -> SBUF bf16.  k-tiles >= 9
     only compute q >= 896.
  F. ctx[q,u] = sum_k Et[k,q] V[k,u] in bf16; denom via N=1 ones-column
     matmul under the same stationary Et; out = ctx * (1/denom).
     q-tiles < 7 stop the k-sum at tile 8.

Nonzero bq/bk are handled exactly by augmenting the contraction with a
9th d-tile: x~ = [x, 1], W~ = [W; b] (separate compiled variant; the
common all-zero-bias case never pays for it).  bv is always applied.
"""

import os
import sys

import numpy as np

for _p in ("/opt/trn_rl_repo", "/opt/pypackages"):
    if _p not in sys.path and os.path.isdir(_p):
        sys.path.append(_p)

import concourse.bass as bass
import concourse.tile as tile
from concourse import bacc, masks, mybir
from concourse.bass import ts
from concourse.bass_utils import run_bass_kernel_spmd

P = 128
B, S, D, U = 8, 2048, 1024, 1024
NCORES = 8
NG = 512
DT, UT, ST, KT = D // P, U // P, S // P, S // P  # 8, 8, 16, 16
QG = S // NG  # 4
SCALE = 1.0 / 32.0  # 1/sqrt(U)

# mask-skip capacity window: with tokens sorted unmasked-first,
# q < QL*P are guaranteed unmasked and k >= KL*P guaranteed masked
# whenever QL*P <= n_unmasked <= KL*P.
QL = 7  # q-tiles fully unmasked (q < 896)
KL = 9  # first fully-masked k-tile (k >= 1152)

F32 = mybir.dt.float32
BF16 = mybir.dt.bfloat16
FP8 = mybir.dt.float8e4
I32 = mybir.dt.int32
AF = mybir.ActivationFunctionType
ALU = mybir.AluOpType
PM_DR = mybir.MatmulPerfMode.DoubleRow
T1S = 64.0  # scores are accumulated as 64*s so T1 fits e4m3 normal range

_cache = {}
last_results = None


def _emit(tc, aug: bool, skip: bool):
    nc = tc.nc
    DTE = DT + 1 if aug else DT  # d-tiles incl. bias augmentation
    DA = DTE * P  # augmented d extent (free dim of M rows)
    # fp8 half for phase E (non-aug only): score columns q >= 1024 are
    # computed as pure fp8e4 DoubleRow chains (2x PE throughput), columns
    # q < 1024 as pure bf16 chains — no perf-mode switch inside a PSUM
    # accumulation group (switching mid-group wedges the exec unit).
    # T1 is scaled by T1S=64 into e4m3 normal range, so the whole scores
    # accumulation carries a 64x factor that the Exp epilogue removes via
    # its scale argument.  The fp8 lhsT [d2, k-block] is quantized per
    # k-tile on the fly from xT (2-deep ring).
    f8 = not aug
    SH = 0  # first fp8 score column (cols SH..S are fp8, 0..SH bf16)
    # M free-dim groups: [offset, width] pairs
    MG = [(0, NG), (NG, NG)] + ([(2 * NG, P)] if aug else [])

    x_d = nc.dram_tensor("x", [S, D], F32, kind="ExternalInput").ap()
    m_d = nc.dram_tensor("mask", [1, S], I32, kind="ExternalInput").ap()
    w_d = {
        "q": nc.dram_tensor("wq", [D, U], F32, kind="ExternalInput").ap(),
        "k": nc.dram_tensor("wk", [D, U], F32, kind="ExternalInput").ap(),
        "v": nc.dram_tensor("wv", [D, U], F32, kind="ExternalInput").ap(),
    }
    bq_d = nc.dram_tensor("bq", [1, U], F32, kind="ExternalInput").ap()
    bk_d = nc.dram_tensor("bk", [1, U], F32, kind="ExternalInput").ap()
    bv_d = nc.dram_tensor("bv", [1, U], F32, kind="ExternalInput").ap()
    out_d = nc.dram_tensor("out", [S, U], F32, kind="ExternalOutput").ap()

    with tc.tile_pool(name="main", bufs=1) as main:
        # ---------------- small persistent tensors ----------------
        identity = main.tile([P, P], BF16, tag="ident", name="identity")
        masks.make_identity(nc, identity[:])

        rows = main.tile([1, S + U + P], BF16, tag="rows", name="rows")
        m_row = rows[:, 0:S]
        bv_row = rows[:, S : S + U]
        ones_row = rows[:, S + U : S + U + P]
        nc.vector.memset(ones_row, 1.0)

        ones_col = main.tile([P, 1], BF16, tag="onec", name="ones_col")
        nc.vector.memset(ones_col[:], 1.0)

        consts = main.tile([P, KT + 2 * UT], F32, tag="consts", name="consts")
        c_cols = consts[:, 0:KT]  # -10000*(1-m) per k partition
        bq_cols = consts[:, KT : KT + UT]
        bk_cols = consts[:, KT + UT : KT + 2 * UT]

        m_bcast = main.tile([P, S], BF16, tag="mb", name="m_bcast")
        bv_bcast = main.tile([P, U], BF16, tag="bvb", name="bv_bcast")

        # small HWDGE loads up front on the sync queue
        mk_i32 = main.tile([P, KT], I32, tag="mk", name="mk_i32")
        nc.sync.dma_start(mk_i32[:], m_d.rearrange("a (t p) -> p (a t)", p=P))
        if aug:
            nc.sync.dma_start(bq_cols, bq_d.rearrange("a (j p) -> p (a j)", p=P))
            nc.sync.dma_start(bk_cols, bk_d.rearrange("a (j p) -> p (a j)", p=P))

        # transient mask row load; its slot is later reused by the E-phase
        # output staging tile (tag "mi")
        m_i32 = main.tile([1, S], I32, tag="mi", name="m_i32")
        nc.sync.dma_start(m_i32[:], m_d)
        nc.vector.tensor_copy(m_row, m_i32[:])
        # c = m*B - B  -> 0 where m==1, -B where m==0  (B carries the T1S
        # scores scaling in the fp8 variant)
        big = 10000.0 * (T1S if f8 else 1.0)
        nc.vector.tensor_scalar(c_cols, mk_i32[:], big, -big, ALU.mult, ALU.add)
        # bv staging reuses the mask-row slot (sequential, WAR-ordered)
        bv_f32 = main.tile([1, U], F32, tag="mi", name="bv_f32")
        nc.sync.dma_start(bv_f32[:], bv_d)
        nc.vector.tensor_copy(bv_row, bv_f32[:])

        # xT holds x^T [d,s] (tiles 0..7) + optional all-ones aug row tile
        xT = main.tile([P, DTE, S], BF16, tag="xT", name="xT")
        # M shares its 64KB slot with Et (M dies when T1T completes,
        # Et is born in phase E)
        M_sb = main.tile([P, DTE, DA], BF16, tag="met", name="M_sb")

        # x chunk emission: f32 via HWDGE (even chunks on the SP queue,
        # odd on the ACT queue behind Wq), ACT cast, PE transpose.
        def emit_x_chunk(st):
            xf = main.tile([P, D], F32, tag="xf", bufs=3, name=f"xf_{st}")
            eng = nc.sync if st % 2 == 0 else nc.scalar
            eng.dma_start(xf[:], x_d[ts(st, P), :])
            xs = main.tile([P, D], BF16, tag="xs", bufs=3, name=f"x_{st}")
            nc.scalar.activation(xs[:], xf[:], AF.Copy)
            px = psTx.tile([P, DT, P], BF16, tag="pt", name="ps_xT")
            for dt in range(DT):
                nc.tensor.matmul(
                    px[:, dt, :],
                    lhsT=xs[:, ts(dt, P)],
                    rhs=identity[:],
                    is_transpose=True,
                    start=(dt == 0),
                    stop=(dt == DT - 1),
                    skip_group_check=True,
                )
            nc.vector.tensor_copy(xT[:, 0:DT, ts(st, P)], px[:])

        # ---------------- phase A: Wq/Wk load + transpose ----------------
        wqT = {}
        # psTx hosts all PE-transpose outputs (W in phase A, x in B/C);
        # 2 banks, coexists with psM (6 banks) and psT1 (4 banks)
        psTx_cm = tc.tile_pool(name="psTx", bufs=2, space="PSUM")
        psTx = psTx_cm.__enter__()
        with tc.tile_pool(name="wpool", bufs=1) as wpool:
            for which in ("q", "k"):
                wqT[which] = wpool.tile(
                    [P, UT, DA], BF16, tag=f"w{which}T", name=f"w{which}T"
                )

            def emit_w_chunk(which, dt):
                src = w_d[which].rearrange("(t p) u -> p t u", p=P)[:, dt, :]
                stage = wpool.tile(
                    [P, U], BF16, tag="wstage", bufs=4, name=f"w{which}_{dt}"
                )
                if which == "k":
                    nc.gpsimd.dma_start(stage[:], src)  # f32 -> bf16 (SWDGE)
                else:
                    # Wq rides the two HWDGE queues as f32, pair-wise
                    # (0,1 -> ACT queue which spins up first, 2,3 -> SP, ...)
                    # so neither queue serializes 4MB ahead of the x chunks
                    # and the wf32 ring drains in arrival order; cast on ACT
                    wf = wpool.tile([P, U], F32, tag="wf32", bufs=4, name=f"wf_{dt}")
                    eng = nc.scalar if (dt // 2) % 2 == 0 else nc.sync
                    eng.dma_start(wf[:], src)
                    nc.scalar.activation(stage[:], wf[:], AF.Copy)
                pt = psTx.tile([P, UT, P], BF16, tag="pt", name="ps_wT")
                for ut in range(UT):
                    nc.tensor.matmul(
                        pt[:, ut, :],
                        lhsT=stage[:, ts(ut, P)],
                        rhs=identity[:],
                        is_transpose=True,
                        start=(ut == 0),
                        stop=(ut == UT - 1),
                        skip_group_check=True,
                    )
                nc.vector.tensor_copy(wqT[which][:, :, ts(dt, P)], pt[:])

            # ---------------- phase B: M = Wq Wk^T / 32 ----------------
            # W chunks are emitted interleaved with M's accumulation blocks
            # (g-major) so each block's operands have just arrived and the
            # tensor queue never parks behind a not-yet-loaded chunk.
            # x chunks 0..7 are transposed between blocks to fill the
            # DMA-bound start of the kernel.
            if aug:
                d1_passes = [[0, 1], [2, 3], [4, 5], [6, 7], [8]]
                m_bufs = {NG: 4, P: 2}
            else:
                d1_passes = [[0, 1, 2, 3], [4, 5, 6, 7]]
                m_bufs = {NG: 6}

            with tc.tile_pool(name="psM", bufs=1, space="PSUM") as psM:

                def m_block(gi, d1_list):
                    go, gw = MG[gi]
                    pm = {}
                    for d1t in d1_list:
                        pm[d1t] = psM.tile(
                            [P, gw], F32, tag=f"m{gw}", bufs=m_bufs[gw], name="ps_M"
                        )
                    for ut in range(UT):
                        for d1t in d1_list:
                            nc.tensor.matmul(
                                pm[d1t][:],
                                lhsT=wqT["q"][:, ut, ts(d1t, P)],
                                rhs=wqT["k"][:, ut, go : go + gw],
                                start=(ut == 0),
                                stop=(ut == UT - 1),
                            )
                    for d1t in d1_list:
                        nc.vector.tensor_scalar_mul(
                            M_sb[:, d1t, go : go + gw], pm[d1t][:], SCALE
                        )

                if aug:
                    # rare path: plain order — all chunks, fixups, then M
                    for dt in range(DT):
                        emit_w_chunk("k", dt)
                        emit_w_chunk("q", dt)
                    for which, bcols in (("q", bq_cols), ("k", bk_cols)):
                        nc.vector.memset(wqT[which][:, :, D : DA], 0.0)
                        for ut in range(UT):
                            nc.vector.tensor_copy(
                                wqT[which][:, ut, D : D + 1], bcols[:, ut : ut + 1]
                            )
                    for gi in range(len(MG)):
                        for d1_list in d1_passes:
                            m_block(gi, d1_list)
                else:
                    # wq (HWDGE) lands a few us before wk (SWDGE ring
                    # startup) — transpose it first so the PE starts early
                    for dt in range(4):
                        emit_w_chunk("q", dt)
                        emit_w_chunk("k", dt)
                    emit_x_chunk(0)
                    emit_x_chunk(2)
                    m_block(0, d1_passes[0])  # needs wk0-3 + wq0-3
                    emit_x_chunk(4)
                    emit_x_chunk(6)
                    for dt in range(4, 8):
                        emit_w_chunk("q", dt)
                    emit_x_chunk(1)
                    emit_x_chunk(3)
                    m_block(0, d1_passes[1])  # + wq4-7
                    for dt in range(4, 8):
                        emit_w_chunk("k", dt)
                    emit_x_chunk(8)
                    emit_x_chunk(10)
                    m_block(1, d1_passes[0])  # + wk4-7
                    emit_x_chunk(5)
                    emit_x_chunk(7)
                    m_block(1, d1_passes[1])
                    # chunks {0..8, 10} emitted; {9, 11, 12..15} pend for C

        if aug:
            # aug x-tile: partition 0 = ones row, rest zero
            nc.vector.memset(xT[0:1, DT, :], 1.0)
            nc.vector.memset(xT[1:P, DT, :], 0.0)

        with tc.tile_pool(name="late", bufs=1) as late:
            if f8:
                # score cols 0:SH read bf16 T1 (x64), cols SH:S e4m3 (x64)
                t1T8 = late.tile([P, DT, S - SH], FP8, tag="t1t8", name="t1T8")
                t1T = (
                    late.tile([P, DT, SH], BF16, tag="t1t", name="t1T")
                    if SH > 0
                    else None
                )
            else:
                t1T = late.tile([P, DTE, S], BF16, tag="t1t", name="t1T")
            v_sb = late.tile([P, ST, U], BF16, tag="v", name="v_sb")

            # -------- phase C: remaining x transposes + T1T --------
            # each qg's x chunks are fully emitted before its first T1T
            # matmul (aug: v2-style next-qg pacing; non-aug: the chunks
            # left over from phase B, two per qg)
            pending = [] if aug else [9, 11, 12, 13, 14, 15]
            with tc.tile_pool(name="psT1", bufs=6, space="PSUM") as psT1:
                if aug:
                    for st in range(4):
                        emit_x_chunk(st)
                for qg in range(QG):
                    for d2t in range(DTE):
                        if aug:
                            if qg < QG - 1 and d2t < 4:
                                emit_x_chunk(4 * (qg + 1) + d2t)
                        elif pending and d2t % 3 == 0:
                            emit_x_chunk(pending.pop(0))
                        pt1 = psT1.tile([P, NG], F32, tag="t1", name="ps_t1")
                        for d1t in range(DTE):
                            nc.tensor.matmul(
                                pt1[:],
                                lhsT=M_sb[:, d1t, ts(d2t, P)],
                                rhs=xT[:, d1t, ts(qg, NG)],
                                start=(d1t == 0),
                                stop=(d1t == DTE - 1),
                            )
                        if not f8:
                            nc.vector.tensor_copy(t1T[:, d2t, ts(qg, NG)], pt1[:])
                        elif qg * NG < SH:
                            nc.vector.tensor_scalar_mul(
                                t1T[:, d2t, ts(qg, NG)], pt1[:], T1S
                            )
                        else:
                            nc.vector.tensor_scalar_mul(
                                t1T8[:, d2t, qg * NG - SH : (qg + 1) * NG - SH],
                                pt1[:],
                                T1S,
                            )
            psTx_cm.__exit__(None, None, None)

            # bv + mask-row broadcasts (placed here, NOT at kernel start:
            # heading the PE stream with them gates the whole engine on
            # the small-load queue's startup, measured +2.4us)
            wv_ap = w_d["v"].rearrange("(t p) u -> p t u", p=P)
            with tc.tile_pool(name="psI1", bufs=2, space="PSUM") as psI1:
                for ug in range(2):
                    pi = psI1.tile([P, NG], F32, tag="i1", name="ps_bv")
                    nc.tensor.matmul(
                        pi[:], lhsT=ones_row[:, 0:P], rhs=bv_row[:, ts(ug, NG)]
                    )
                    nc.vector.tensor_copy(bv_bcast[:, ts(ug, NG)], pi[:])
                for qg in range(QG):
                    pi = psI1.tile([P, NG], F32, tag="i1", name="ps_m")
                    nc.tensor.matmul(
                        pi[:], lhsT=ones_row[:, 0:P], rhs=m_row[:, ts(qg, NG)]
                    )
                    nc.vector.tensor_copy(m_bcast[:, ts(qg, NG)], pi[:])

            # ---------------- phase D: V = x Wv + bv ----------------
            # uq-outer quarters: Wv loaded per 256-col quarter (SWDGE,
            # casts) in a 2-deep ring so the next quarter streams while
            # the current one computes
            NQ = U // 4  # 256
            with tc.tile_pool(name="psV", bufs=6, space="PSUM") as psV:
                wvq = {}

                def load_wv_quarter(uq):
                    t = late.tile([P, DT, NQ], BF16, tag="wv", bufs=2, name="wvq")
                    nc.gpsimd.dma_start(t[:], wv_ap[:, :, NQ * uq : NQ * (uq + 1)])
                    wvq[uq] = t

                load_wv_quarter(0)
                load_wv_quarter(1)
                for uq in range(4):
                    for st in range(ST):
                        pv = psV.tile([P, NQ], F32, tag="v", name="ps_v")
                        for dt in range(DT):
                            nc.tensor.matmul(
                                pv[:],
                                lhsT=xT[:, dt, ts(st, P)],
                                rhs=wvq[uq][:, dt, :],
                                start=(dt == 0),
                                stop=(dt == DT - 1),
                            )
                        nc.vector.tensor_tensor(
                            v_sb[:, st, NQ * uq : NQ * (uq + 1)],
                            pv[:],
                            bv_bcast[:, NQ * uq : NQ * (uq + 1)],
                            ALU.add,
                        )
                    if uq + 2 < 4:
                        load_wv_quarter(uq + 2)

            # -------- phase E: scoresT = xT^T T1T, mask, exp --------
            # phases E and F share one PSUM ring (tag "sc") so F's first
            # accumulations start while E's last epilogues drain
            et_sb = main.tile([P, KT, S], BF16, tag="met", name="et_sb")
            full_cols = [(qg * NG, (qg + 1) * NG) for qg in range(QG)]

            # per-k-tile fp8 quantization of xT's [d2, k-block] for the
            # DoubleRow lhsT, in a 2-deep ring one k-tile ahead of use
            x8blk = {}

            def quant_x8(kt):
                t = main.tile([P, DT, P], FP8, tag="x8b", bufs=2, name="x8blk")
                nc.vector.tensor_copy(t[:], xT[:, 0:DT, ts(kt, P)])
                x8blk[kt] = t

            with tc.tile_pool(name="psDE", bufs=8, space="PSUM") as psDE:
                if f8:
                    quant_x8(0)
                for kt in range(KT):
                    if f8 and kt + 1 < KT:
                        quant_x8(kt + 1)
                    if skip and kt >= KL:
                        # masked k-tile: only q >= 896 can see it
                        cols = [(QL * P, 2 * NG)] + full_cols[2:]
                    else:
                        cols = full_cols
                    pss = [
                        psDE.tile([P, NG], F32, tag="sc", name="ps_sc")
                        for _ in cols
                    ]
                    for ci, (lo, hi) in enumerate(cols):
                        pv = pss[ci][:, 0 : hi - lo]
                        if f8 and lo >= SH:
                            # pure-fp8 DoubleRow chain over 4 d2-pairs
                            for j in range(DT // 2):
                                nc.tensor.matmul(
                                    pv,
                                    lhsT=x8blk[kt][:, 2 * j : 2 * j + 2, :],
                                    rhs=t1T8[:, 2 * j : 2 * j + 2, lo - SH : hi - SH],
                                    start=(j == 0),
                                    stop=(j == DT // 2 - 1),
                                    perf_mode=PM_DR,
                                )
                        else:
                            for d2t in range(DTE):
                                nc.tensor.matmul(
                                    pv,
                                    lhsT=xT[:, d2t, ts(kt, P)],
                                    rhs=t1T[:, d2t, lo:hi],
                                    start=(d2t == 0),
                                    stop=(d2t == DTE - 1),
                                )
                    for ci, (lo, hi) in enumerate(cols):
                        pv = pss[ci][:, 0 : hi - lo]
                        if not (skip and kt < QL):
                            # scores += c_k * m_q  (rank-1 mask term on DVE;
                            # identically 0 for fully-unmasked k-tiles)
                            nc.vector.scalar_tensor_tensor(
                                pv,
                                m_bcast[:, lo:hi],
                                c_cols[:, kt : kt + 1],
                                pv,
                                ALU.mult,
                                ALU.add,
                            )
                        # psum holds T1S*scores in the fp8 variant
                        nc.scalar.activation(
                            et_sb[:, kt, lo:hi], pv, AF.Exp,
                            scale=(1.0 / T1S) if f8 else 1.0,
                        )

                # -------- phase F: ctx = Et^T V, denom, normalize --------
                for qt in range(KT):
                    ktmax = KL if (skip and qt < QL) else KT
                    pc = [
                        psDE.tile([P, NG], F32, tag="sc", name="ps_ctx")
                        for _ in range(2)
                    ]
                    den = psDE.tile([P, NG], F32, tag="sc", name="ps_den")[:, 0:1]
                    for kt in range(ktmax):
                        lhsT = et_sb[:, kt, ts(qt, P)]
                        first, last = kt == 0, kt == ktmax - 1
                        for ug in range(2):
                            nc.tensor.matmul(
                                pc[ug][:],
                                lhsT=lhsT,
                                rhs=v_sb[:, kt, ts(ug, NG)],
                                start=first,
                                stop=last,
                            )
                        nc.tensor.matmul(
                            den, lhsT=lhsT, rhs=ones_col[:], start=first, stop=last
                        )
                    recip = main.tile([P, 1], F32, tag="recip", bufs=2, name="recip")
                    nc.vector.reciprocal(recip[:], den)
                    o = main.tile([P, U], F32, tag="mi", bufs=1, name="o_sb")
                    for ug in range(2):
                        nc.vector.tensor_scalar_mul(o[:, ts(ug, NG)], pc[ug][:], recip[:])
                    if qt >= KT - 2:
                        # drain the tail on both queues
                        nc.sync.dma_start(out_d[ts(qt, P), 0:NG], o[:, 0:NG])
                        nc.scalar.dma_start(out_d[ts(qt, P), NG:U], o[:, NG:U])
                    else:
                        eng = nc.sync if qt % 2 == 0 else nc.scalar
                        eng.dma_start(out_d[ts(qt, P), :], o[:])


def _build(aug: bool, skip: bool):
    key = ("nc", aug, skip)
    if key in _cache:
        return _cache[key]
    nc = bacc.Bacc("TRN2", target_bir_lowering=False, debug=False, num_devices=NCORES)
    with tile.TileContext(nc) as tc:
        _emit(tc, aug, skip)
    nc.compile()
    _cache[key] = nc
    return nc


def kernel(x, mask, Wq, bq, Wk, bk, Wv, bv):
    global last_results
    bqr = np.ascontiguousarray(bq, dtype=np.float32).reshape(1, U)
    bkr = np.ascontiguousarray(bk, dtype=np.float32).reshape(1, U)
    bvr = np.ascontiguousarray(bv, dtype=np.float32).reshape(1, U)
    aug = bool(np.any(bqr) or np.any(bkr))

    mask_i = np.ascontiguousarray(mask, dtype=np.int32).reshape(B, S)
    # sort tokens unmasked-first per batch; exact-zero score blocks can
    # then be skipped statically when every batch's unmasked count fits
    # the [QL*P, KL*P] capacity window
    perms = np.argsort(1 - mask_i, axis=1, kind="stable")
    n_u = mask_i.sum(axis=1)
    skip = bool(np.all((n_u >= QL * P) & (n_u <= KL * P)))
    if not skip:
        perms = np.tile(np.arange(S), (B, 1))

    nc = _build(aug, skip)
    wq = np.ascontiguousarray(Wq, dtype=np.float32)
    wk = np.ascontiguousarray(Wk, dtype=np.float32)
    wv = np.ascontiguousarray(Wv, dtype=np.float32)
    in_maps = []
    for b in range(B):
        in_maps.append(
            {
                "x": np.ascontiguousarray(x[b][perms[b]], dtype=np.float32),
                "mask": np.ascontiguousarray(mask_i[b][perms[b]]).reshape(1, S),
                "wq": wq,
                "wk": wk,
                "wv": wv,
                "bq": bqr,
                "bk": bkr,
                "bv": bvr,
            }
        )
    res = run_bass_kernel_spmd(
        nc,
        in_maps,
        core_ids=list(range(NCORES)),
        trace=bool(int(os.environ.get("KERNEL_TRACE", "0"))),
        tmpdir=os.environ.get("KERNEL_TRACE_DIR"),
    )
    last_results = res
    out = np.empty((B, S, U), dtype=np.float32)
    for b in range(B):
        out[b][perms[b]] = res.results[b]["out"]
    return out
